# revision 1
# baseline (speedup 1.0000x reference)
"""Trainium2 Bass kernel for an autoregressive free-run rollout of a small MLP.

Model (per reference.py):
    B=64, C_IN=4, C_OUT=4, T=1024, H=512, RF=32, io_delay=1
    step t:  h = relu(Wu @ u_win[t] + Wy @ y_win[t] + b1);  y[t] = W2 @ h + b2
    u_win[t] = delayed-input window (recurrence-independent)
    y_win[t] = previous RF outputs (sequential dependency)

Sharding: data-parallel over batch across 8 cores (B_LOC=8/core), weights
replicated, zero inter-core communication.

Device algorithm (per core, fully unrolled over T).  The serial recurrence
is restructured so each step has only TWO engine hops (ACT -> PE -> ACT):

  - u-contribution ("u_proj") for every step is batch-matmul'ed into PSUM
    regions ahead of time (6 banks x 16 steps; refilled in the shadow).
  - The delay-1 feedback (the only part that cannot be precomputed early)
    is applied as a FOLDED matrix Wh1 = Wy[:,:,RF-1] @ W2 (512x512, rank 4)
    acting directly on h_t: region[t+1] += Wh1^T h_t.  16 chunk matmuls on
    PE, no PSUM round-trip through y.
  - Serial chain per step t:   relu_t (ACT, region->h) -> fold (PE, 16 mm
    into region t+1) -> relu_{t+1} ...
  - Off the chain each step: MM2 (4 padded chunk matmuls + 1 merge matmul)
    emits the y quadrant in PSUM; DVE copies it into the y window (ycb) and
    into the output staging buffer (yhist); a 4-matmul gather with
    pre-rotated, delay-1-slot-ZEROED weights accumulates the d>=2 window
    contribution into region t+2 (~1.5 cycles of slack).
  - y window lives in SBUF as (128, 8): partition 4s+o = slot s, channel o.
    Slot->delay mapping rotates with t; 32 pre-rotated copies of Wy (with
    the d=1 slot zeroed) handle it.
  - Output staging y_hist (128, 8*T): quadrant stripes; host extracts the 4
    meaningful lanes per step and reassembles (B, C_OUT, T).
"""

import numpy as np

import concourse.bacc as bacc
import concourse.mybir as mybir
from concourse import bass_utils
from concourse.tile import TileContext

# Problem constants (hardcoded per contract).
B_FULL, C_IN, C_OUT, T, H, RF = 64, 4, 4, 1024, 512, 32
IO_DELAY = 1
N_CORES = 8
B_LOC = B_FULL // N_CORES          # 8
NCH = H // 128                     # 4 H-chunks
NREG = 16                          # psum regions (steps) per bank
NBANK = 6                          # psum banks holding u_proj preactivations
NYQ = 1                            # psum tiles for the y quadrant
F32 = mybir.dt.float32

_cache = {}
FB_BF16 = True  # bf16 feedback path (Wy, y window, W2, h); fp32 u-path
ACT_RELU = False  # relu engine: ACT (True) or DVE (False)
POOL_COPY = False  # ylast copy engine: Pool/gpsimd (True) or DVE (False)
SPLIT_RELU = False  # split relu DVE(c0,c1) + ACT(c2,c3)
NFILL = 0  # PE-warming filler matmuls per step (p-state hold)


def _build(T_steps, b2_any=False, fb_bf16=False, reps=1):
    """Build the Bacc program (SPMD, identical on all cores)."""
    nc = bacc.Bacc("TRN2", target_bir_lowering=False, debug=False,
                   num_devices=N_CORES)

    DT = mybir.dt.bfloat16 if fb_bf16 else F32
    d_u = nc.dram_tensor("u_lay", [128, T_steps * B_LOC], F32,
                         kind="ExternalInput").ap()
    d_wyrot = nc.dram_tensor("wy_rot", [128, 32 * NCH * 128], DT,
                             kind="ExternalInput").ap()
    d_wut = nc.dram_tensor("wu_t", [128, H], F32, kind="ExternalInput").ap()
    d_w2z = nc.dram_tensor("w2_zpad", [128, NCH * 128], DT,
                           kind="ExternalInput").ap()
    d_wy1t = nc.dram_tensor("wy1_t", [128, NCH * 128], DT,
                            kind="ExternalInput").ap()
    d_plc = nc.dram_tensor("place", [128, 8 * 32], DT,
                           kind="ExternalInput").ap()
    d_sg = nc.dram_tensor("s_merge", [128, 8 * 32], DT,
                          kind="ExternalInput").ap()
    d_b2m = nc.dram_tensor("b2_mask", [128, 8], F32,
                           kind="ExternalInput").ap()
    d_wyb = nc.dram_tensor("wy1b2", [128, NCH * 128], F32,
                           kind="ExternalInput").ap()
    d_out = nc.dram_tensor("y_hist", [128, B_LOC * (T_steps // 8)], F32,
                           kind="ExternalOutput").ap()

    n_windows = T_steps // NREG
    assert T_steps % NREG == 0

    with TileContext(nc) as tc:
        with (
            tc.tile_pool(name="const", bufs=1) as cpool,
            tc.tile_pool(name="hp", bufs=8) as hpool,
            tc.tile_pool(name="ph", bufs=1, space="PSUM") as ppool,
            tc.tile_pool(name="py", bufs=1, space="PSUM") as pypool,
        ):
            U = cpool.tile([128, T_steps * B_LOC], F32, tag="U")
            WyR = cpool.tile([128, 32 * NCH * 128], DT, tag="WyR")
            WuT = cpool.tile([128, H], F32, tag="WuT")
            W2Z = cpool.tile([128, NCH * 128], DT, tag="W2Z")
            WY1T = cpool.tile([128, NCH * 128], DT, tag="WY1T")
            PLC = cpool.tile([128, 8 * 32], DT, tag="PLC")
            SG = cpool.tile([128, 8 * 32], DT, tag="SG")
            B2M = cpool.tile([128, 8], F32, tag="B2M")
            WYB = cpool.tile([128, NCH * 128], F32, tag="WYB")
            ONES = cpool.tile([128, B_LOC], DT, tag="ONES")
            ycb = cpool.tile([128, B_LOC], DT, tag="ycb")
            ylast = cpool.tile([128, B_LOC], DT, tag="ylast")
            yhist = cpool.tile([128, B_LOC * (T_steps // 8)], F32,
                               tag="yhist")

            # Split big input DMAs: the chain's first steps depend only on
            # the first slices, so compute starts ~30us earlier.
            upiece = T_steps * B_LOC // 4
            for i in range(4):
                nc.sync.dma_start(U[:, i * upiece:(i + 1) * upiece],
                                  d_u[:, i * upiece:(i + 1) * upiece])
            for i in range(4):
                nc.sync.dma_start(WyR[:, i * 4096:(i + 1) * 4096],
                                  d_wyrot[:, i * 4096:(i + 1) * 4096])
            nc.sync.dma_start(WuT[:], d_wut)
            nc.sync.dma_start(W2Z[:], d_w2z)
            nc.sync.dma_start(WY1T[:], d_wy1t)
            nc.sync.dma_start(PLC[:], d_plc)
            nc.sync.dma_start(SG[:], d_sg)
            nc.sync.dma_start(B2M[:], d_b2m)
            if b2_any:
                nc.sync.dma_start(WYB[:], d_wyb)
                nc.gpsimd.memset(ONES[:], 1.0)
            pbank = [ppool.tile([128, NREG * 32], F32, tag=f"pb{w}",
                                name=f"pb{w}") for w in range(NBANK)]
            pyq = [pypool.tile([128, B_LOC], F32, tag=f"py{i}",
                               name=f"py{i}") for i in range(NYQ)]
            py0 = pypool.tile([128, 2 * B_LOC], F32, tag="pyy", name="pyy")

            # Steps are split by PARITY across two bank groups (even steps
            # in banks 0-2, odd in banks 3-5) so that relu(t+1) and the
            # gather into region t+2 touch DIFFERENT tiles -- the tile
            # framework tracks deps per tile, and sharing a tile between
            # consecutive steps puts the off-chain gather onto the serial
            # chain.  Each bank holds 16 same-parity steps (a 32-step
            # "window" per parity), cycling over 3 banks per parity.
            def region_of(t):
                p = t % 2
                bank = pbank[p * 3 + (t // 32) % 3]
                reg = (t % 32) // 2
                return bank[:, reg * 32:(reg + 1) * 32]

            def preload(p, w32, chunk):
                """u_proj matmul (one H-chunk) for the 16 parity-p steps of
                32-step window w32 into bank p*3 + w32%3.

                start=True clears has_written for the WHOLE bank, so only
                chunk 0 starts; chunks 1-3 write with cleared bits (fresh
                overwrite) and set them, letting later matmuls accumulate."""
                bank = pbank[p * 3 + w32 % 3]
                # u_lay is parity-permuted on the host: window w32's 16
                # parity-p steps occupy one contiguous 128-column slice.
                base = (w32 * 32 + p * 16) * B_LOC
                rhs = U[:, base:base + 16 * B_LOC]
                out = bank[:].rearrange("p (r cb) -> p r cb", cb=32)[
                    :, :, chunk * B_LOC:(chunk + 1) * B_LOC]
                nc.tensor.matmul(out, WuT[:, chunk * 128:(chunk + 1) * 128],
                                 rhs, start=(chunk == 0), stop=False,
                                 skip_group_check=True)

            def gather(tdst):
                """d>=2 window contribution for region tdst (4 chunk mm).

                Uses rotation rho = tdst%32, whose pre-rotated weights have
                the d=1 slot zeroed; requires ycb to hold y_{tdst-2}."""
                rho = tdst % 32
                region = region_of(tdst)
                for c in range(NCH):
                    nc.tensor.matmul(
                        region[:, c * B_LOC:(c + 1) * B_LOC],
                        WyR[:, (rho * NCH + c) * 128:(rho * NCH + c + 1) * 128],
                        ycb[:],
                        start=False, stop=False, skip_group_check=True)

            n_win32 = T_steps // 32
            assert T_steps % 32 == 0
            for rep in range(reps):
              nc.gpsimd.memset(ycb[:], 0.0)
              for w32 in range(min(3, n_win32)):
                for p in range(2):
                    for c in range(NCH):
                        preload(p, w32, c)

              h_prev = None
              for t in range(T_steps):
                  s = t % 32
                  q = s // 8
                  g = s % 8
                  region = region_of(t)

                  # CHAIN 1/4 -- relu -> SBUF h (DVE; b1 == 0 here).  The
                  # region was completed by step t-1's MM1b (stop=True).
                  h = hpool.tile([128, NCH * B_LOC], DT, tag="h")
                  if SPLIT_RELU:
                      # chunks 0-1 on DVE, chunks 2-3 on ACT, in parallel;
                      # MM2 consumes chunk c as soon as its half is ready.
                      nc.vector.tensor_relu(h[:, 0:2 * B_LOC],
                                            region[:, 0:2 * B_LOC])
                      nc.scalar.activation(h[:, 2 * B_LOC:4 * B_LOC],
                                           region[:, 2 * B_LOC:4 * B_LOC],
                                           mybir.ActivationFunctionType.Relu)
                  elif ACT_RELU:
                      nc.scalar.activation(h[:], region,
                                           mybir.ActivationFunctionType.Relu)
                  else:
                      nc.vector.tensor_relu(h[:], region)

                  # CHAIN 2/4 -- MM2: y_t = W2 @ h_t into py0 partitions
                  # 0..3 (W2 zero-padded to 32 rows so the full 32-partition
                  # group is defined for the copy below).
                  for c in range(NCH):
                      nc.tensor.matmul(
                          py0[:, 0:B_LOC],
                          W2Z[:, c * 128:(c + 1) * 128],
                          h[:, c * B_LOC:(c + 1) * B_LOC],
                          start=(c == 0), stop=(c == NCH - 1),
                          skip_group_check=True)

                  # CHAIN 3/4 -- y_t PSUM -> SBUF copy.
                  if POOL_COPY:
                      nc.gpsimd.tensor_copy(ylast[:, :], py0[:, 0:B_LOC])
                  else:
                      nc.vector.tensor_copy(ylast[:, :], py0[:, 0:B_LOC])

                  # PE-warming fillers: harmless matmuls into a scratch
                  # column range of the pyy bank (never read), keeping the
                  # Tensor engine's p-state ramped between chain bursts.
                  if h_prev is not None:
                      for f in range(NFILL):
                          nc.tensor.matmul(
                              py0[:, B_LOC:2 * B_LOC],
                              W2Z[:, (f % NCH) * 128:(f % NCH + 1) * 128],
                              h_prev[:, (f % NCH) * B_LOC:
                                     (f % NCH + 1) * B_LOC],
                              start=False, stop=False,
                              skip_group_check=True)
                  h_prev = h

                  # CHAIN 4/4 -- d=1 feedback: region[t+1] += Wy_1 @ y_t.
                  # K=4 matmuls (4 rows of weights); last carries stop=True
                  # so the next relu sees a completed accumulation group.
                  if t + 1 < T_steps:
                      nregion = region_of(t + 1)
                      if b2_any:
                          # missing d=1 constant: region[t+1] += Wy_1 @ b2
                          for cp in range(NCH):
                              nc.tensor.matmul(
                                  nregion[:, cp * B_LOC:(cp + 1) * B_LOC],
                                  WYB[0:1, cp * 128:(cp + 1) * 128],
                                  ONES[0:1, :],
                                  start=False, stop=False,
                                  skip_group_check=True)
                      for cp in range(NCH):
                          nc.tensor.matmul(
                              nregion[:, cp * B_LOC:(cp + 1) * B_LOC],
                              WY1T[0:4, cp * 128:(cp + 1) * 128],
                              ylast[0:4, :],
                              start=False, stop=(cp == NCH - 1),
                              skip_group_check=True)

                  # OFF CHAIN -- y window quadrant rebuild in PSUM: place
                  # y_t at lanes 4g..4g+3 (K=4) + merge re-emitting the
                  # quadrant's other 7 slots from ycb (K=32).
                  ysc = pyq[0]
                  oquad = ysc[32 * q:32 * (q + 1), :]
                  nc.tensor.matmul(
                      oquad,
                      PLC[:, g * 32:(g + 1) * 32],
                      ylast[:, :],
                      start=True, stop=False,
                      tile_position=(0, 32 * q), skip_group_check=True)
                  nc.tensor.matmul(
                      oquad,
                      SG[32 * q:32 * (q + 1), g * 32:(g + 1) * 32],
                      ycb[32 * q:32 * (q + 1), :],
                      start=False, stop=True,
                      tile_position=(32 * q, 32 * q), skip_group_check=True)

                  # Updated quadrant -> y window (ACT, off the critical
                  # chain).  b2 == 0 here; the masked bias B2M handles
                  # nonzero b2 generically (ycb then already includes b2).
                  if b2_any:
                      nc.scalar.activation(
                          ycb[32 * q:32 * (q + 1), :], oquad,
                          mybir.ActivationFunctionType.Identity,
                          bias=B2M[32 * q:32 * (q + 1), g:g + 1])
                  else:
                      nc.scalar.activation(
                          ycb[32 * q:32 * (q + 1), :], oquad,
                          mybir.ActivationFunctionType.Copy)

                  # After the 8th step of a group the ycb quadrant holds y
                  # for all 8 steps (the merge matmul keeps re-emitting the
                  # older slots), so ONE group copy per 8 steps captures the
                  # output history.  Host undoes the grouped layout.
                  if g == 7 or t == T_steps - 1:
                      grp = t // 8
                      nc.scalar.activation(
                          yhist[32 * q:32 * (q + 1),
                                grp * B_LOC:(grp + 1) * B_LOC],
                          ycb[32 * q:32 * (q + 1), :],
                          mybir.ActivationFunctionType.Copy)

                  # d>=2 gather for region t+2 (needs y_t in ycb, which the
                  # ACT copy above provides; ~1.5 chain cycles of slack).
                  if t + 2 < T_steps:
                      gather(t + 2)

                  # Stream completed yhist columns to DRAM in the shadow of
                  # the chain; only the last piece remains after the loop.
                  if (t + 1) % 128 == 0 or t == T_steps - 1:
                      g0 = (t // 128) * 16 * B_LOC
                      g1 = (t // 8 + 1) * B_LOC
                      nc.sync.dma_start(d_out[:, g0:g1], yhist[:, g0:g1])

                  # Refill: during 32-step window W, emit the 8 preload
                  # matmuls (2 parities x 4 chunks) for window W+2, whose
                  # banks were freed at the end of window W-1.  One matmul
                  # every 4 steps, off the serial chain.
                  if t % 4 == 0:
                      target = t // 32 + 2
                      slot = (t % 32) // 4          # 0..7
                      if 3 <= target < n_win32:
                          preload(slot // 4, target, slot % 4)

    nc.compile()
    return nc


_wcache = {}


def _host_prep(u_core, W1, b1, W2, b2, T_steps, fb_bf16=False):
    """Build per-core input arrays in device layouts (pure layout work).

    Weight layouts are identical across cores (only u differs), so they are
    memoized on a content digest."""
    import hashlib
    wkey = (hashlib.sha1(np.ascontiguousarray(W1).tobytes()
                         + np.ascontiguousarray(W2).tobytes()
                         + np.ascontiguousarray(b2).tobytes()).hexdigest(),
            T_steps, fb_bf16)
    cached = _wcache.get(wkey)
    if cached is not None:
        out = dict(cached)
        out["u_lay"] = _prep_u(u_core, T_steps)
        return out
    H_, CM, RF_ = W1.shape
    Wu = np.ascontiguousarray(W1[:, :C_IN, :]).reshape(H, C_IN * RF)
    Wy = np.ascontiguousarray(W1[:, C_IN:, :])          # (H, C_OUT, RF)

    u_lay = _prep_u(u_core, T_steps)

    # wu_t[ck, j] = Wu[j, ck]
    wu_t = np.ascontiguousarray(Wu.T)                   # (128, 512)

    # wy_rot[(4s+o), (rho*NCH + c)*128 + j'] = Wy[128c+j', o, RF-d(s, rho)]
    # with the d==1 slot (s = rho-1) ZEROED: that term is applied on-chain
    # by the fold matmul instead.
    wy_rot = np.zeros((128, 32 * NCH * 128), np.float32)
    s_idx = np.arange(32)
    for r in range(32):
        d = ((r - s_idx - 1) % 32) + 1                  # delay of slot s at r
        k = RF - d                                      # (32,)
        blk = Wy[:, :, k]                               # (H, C_OUT, 32) [s]
        blk = blk.transpose(2, 1, 0).reshape(128, H)    # rows (s,o), cols j
        blk = blk.copy()
        s_zero = (r - 1) % 32                           # the d=1 slot
        blk[4 * s_zero:4 * s_zero + 4, :] = 0.0
        wy_rot[:, r * NCH * 128:(r + 1) * NCH * 128] = blk

    # w2z[p, c*128 + m] = W2[m, 128c+p] if m < 4 else 0
    w2z = np.zeros((128, NCH * 128), np.float32)
    for c in range(NCH):
        for o in range(C_OUT):
            w2z[:, c * 128 + o] = W2[o, c * 128:(c + 1) * 128]

    # wy1t[o, cp*128 + j'] = Wy[128cp + j', o, RF-1]   (d=1 weights)
    wy1t = np.zeros((128, NCH * 128), np.float32)
    wy1t[:C_OUT, :] = Wy[:, :, RF - 1].T.reshape(C_OUT, H)

    # place[o, g*32 + m] = 1 if m == 4g+o else 0
    place = np.zeros((128, 8 * 32), np.float32)
    for g in range(8):
        for o in range(C_OUT):
            place[o, g * 32 + 4 * g + o] = 1.0

    # s_merge[32q+i, g*32+m] = 1 if i == m and not (4g <= i < 4g+4) else 0
    s_merge = np.zeros((128, 8 * 32), np.float32)
    eye = np.eye(32, dtype=np.float32)
    for g in range(8):
        m = eye.copy()
        m[4 * g:4 * g + 4, :] = 0.0
        for q in range(4):
            s_merge[32 * q:32 * (q + 1), g * 32:(g + 1) * 32] = m

    # b2_mask[32q + i, g] = b2[i - 4g] if 4g <= i < 4g+4 else 0
    b2_mask = np.zeros((128, 8), np.float32)
    for g in range(8):
        for o in range(C_OUT):
            for q in range(4):
                b2_mask[32 * q + 4 * g + o, g] = b2[o]

    # wy1b2[0, cp*128+j'] = sum_o Wy[128cp+j', o, RF-1] * b2[o]
    wy1b2 = np.zeros((128, NCH * 128), np.float32)
    wy1b2[0, :] = (Wy[:, :, RF - 1] @ np.asarray(b2, np.float32)).reshape(H)

    if fb_bf16:
        import ml_dtypes
        bf = ml_dtypes.bfloat16
        wy_rot = wy_rot.astype(bf)
        w2z = w2z.astype(bf)
        wy1t = wy1t.astype(bf)
        place = place.astype(bf)
        s_merge = s_merge.astype(bf)
    _wcache[wkey] = {"wy_rot": wy_rot, "wu_t": wu_t, "w2_zpad": w2z,
                     "wy1_t": wy1t, "place": place, "s_merge": s_merge,
                     "b2_mask": b2_mask, "wy1b2": wy1b2}
    return {"u_lay": u_lay, **_wcache[wkey]}


def _prep_u(u_core, T_steps):
    """u_lay[c*32+k, col(t)*8+b] = u_padded[b, c, t+k], with columns
    parity-permuted per 32-step window: col order = (w32, t%2, (t%32)//2)
    so each preload's 16 same-parity steps are one contiguous slice."""
    u_pad = np.zeros((B_LOC, C_IN, T_steps + RF - 1), np.float32)
    if T_steps > IO_DELAY:
        u_pad[:, :, RF:] = u_core[:, :, :T_steps - IO_DELAY]
    win = np.lib.stride_tricks.sliding_window_view(u_pad, T_steps, axis=2)
    # win[b, c, k, t] = u_pad[b, c, k + t]
    lay = np.ascontiguousarray(
        win.transpose(1, 2, 3, 0).reshape(128, T_steps * B_LOC))
    t_idx = np.arange(T_steps)
    col = (t_idx // 32) * 32 + (t_idx % 2) * 16 + (t_idx % 32) // 2
    perm = np.empty(T_steps, np.int64)
    perm[col] = t_idx                      # column c holds step perm[c]
    lay = lay.reshape(128, T_steps, B_LOC)[:, perm, :]
    return np.ascontiguousarray(lay.reshape(128, T_steps * B_LOC))


def _extract(y_hist, T_steps):
    """y_hist (128, 8*(T/8)) grouped layout -> (B_LOC, C_OUT, T).

    Step t lives at partition 32*q + 4*g + o (q = (t%32)//8, g = t%8),
    columns (t//8)*B_LOC + b."""
    out = np.empty((B_LOC, C_OUT, T_steps), np.float32)
    t_idx = np.arange(T_steps)
    s = t_idx % 32
    rows = 32 * (s // 8) + 4 * (s % 8)                  # (T,)
    cols = (t_idx // 8)[:, None] * B_LOC + np.arange(B_LOC)[None, :]
    for o in range(C_OUT):
        out[:, o, :] = y_hist[(rows + o)[:, None], cols].T
    return out


def kernel(u, W1, b1, W2, b2):
    T_steps = u.shape[2]
    assert not np.asarray(b1).any(), "kernel assumes b1 == 0"
    b2_any = bool(np.asarray(b2).any())
    key = (T_steps, b2_any, FB_BF16)
    if key not in _cache:
        _cache[key] = _build(T_steps, b2_any, FB_BF16)
    nc = _cache[key]

    in_maps = []
    for core in range(N_CORES):
        u_core = np.asarray(u[core * B_LOC:(core + 1) * B_LOC],
                            dtype=np.float32)
        in_maps.append(_host_prep(u_core, np.asarray(W1), np.asarray(b1),
                                  np.asarray(W2), np.asarray(b2), T_steps,
                                  FB_BF16))

    res = bass_utils.run_bass_kernel_spmd(nc, in_maps,
                                          core_ids=list(range(N_CORES)))
    outs = [_extract(res.results[c]["y_hist"], T_steps)
            for c in range(N_CORES)]
    return np.concatenate(outs, axis=0)

